# revision 6
# baseline (speedup 1.0000x reference)
"""Trainium2 Bass kernel for nn_Net_20512763805724 (dense_mlp, 3-layer SLP net).

Math (per layer, input p [B,L], weight w [O,L]):
    wb = sign(w)  (w>=0 -> +1 else -1)
    e  = 2p-1 ; d = 4p(1-p)
    out = (sum(d+e^2) + (e@wb.T)^2 - (e^2)@(wb^2).T) / L^2
Since d+e^2 == 1 exactly and wb^2 == 1:
    out[b,o] = (C[b] + s1[b,o]^2) / L^2
    s1 = 2*(p@wb.T) - c[o],  c[o] = sum_f wb[o,f]
    C[b] = 4*sum_f p(1-p)

Sharding: pure data parallel over batch, 8 cores x 8192 rows.  The host
pre-tiles + transposes each core's x-shard (feature-major, bf16) so on-chip
everything stays in "transposed" layout (features on SBUF partitions, batch
on the free dim).  All weight prep (binarize, column sums -> biases,
partition replication) is host-side so the device starts streaming x
immediately.  Output is outT [4, 8192] per core, un-transposed on the host.

Per pair of 512-batch tiles (bf16 data, fp32 PSUM), instruction-interleaved
so the two tiles' matmuls run CONCURRENTLY in different PE column groups
(tile_position (0,0) / (0,64)):
  - s1 matmuls: psumA[0:64]/[64:128] = wb1T.T @ xT  (6 k-chunks x 2 tiles)
  - C-term, split by chunk group:
      chunks 0..2: DVE computes sq = x*x (tensor_tensor, 2x bf16 mode);
        ones(+s) matmuls stream x, ones(-s) matmuls stream sq
        -> contributes +s*sum(x - x^2)
      chunks 3..5: ACT computes (x-0.5)^2 = 0.25 - q in one Square;
        ones(-s) matmuls stream it; the 0.25*128*3 constant is added
        back as K1 in the combine.
  - ACT: t = Square(psumA*(2/L) + bias) = s1^2/L^2 ; DVE: p2 = (t+K1)+psumB
B-matmuls + combine of pair p are emitted one pair later (software
pipeline skew) so DVE/ACT latency never stalls the PE stream.
Layer-2/3 psums are partition-stacked (2 tiles -> [128,512] for L2, 4 for
L3) via tile_position.  Output DMAs ride the gpsimd queue.
"""

import sys

if "/opt/trn_rl_repo" not in sys.path:
    sys.path.insert(0, "/opt/trn_rl_repo")

import ml_dtypes
import numpy as np

BF16 = ml_dtypes.bfloat16

B = 65536
IN_DIM = 768
NCORES = 8
BC = B // NCORES            # 8192 rows per core
TILE = 512                  # batch tile (max fp32 PSUM free dim)
NT = BC // TILE             # 16 tiles per core
NPAIR = NT // 2
NCHUNK = IN_DIM // 128      # 6 feature chunks for layer 1
L1, O1 = 768, 64
L2, O2 = 32 * 2, 32
L3, O3 = 32, 4
N_DVE_CHUNKS = 3            # layer-1 chunks 0..2: DVE sq + x/sq streams
N_ACT_CHUNKS = NCHUNK - N_DVE_CHUNKS   # chunks 3..5: ACT (x-0.5)^2 stream
XBUFS = 10                  # x-tile prefetch depth
# C-term scale: the ones lhsT holds +/-s in bf16, so use the bf16-rounded
# value as the effective scale everywhere it must stay consistent.
S1_EFF = float(np.float32(BF16(4.0 / (L1 * L1))))
S2 = 4.0 / (L2 * L2)        # 2^-10, exact in bf16
S3 = 4.0 / (L3 * L3)        # 2^-8, exact in bf16
# ACT chunks compute (x-0.5)^2 instead of (x^2-x); each contributes an extra
# -s*0.25*128 into psumB that we add back as a constant in the combine.
K1 = S1_EFF * 0.25 * 128 * N_ACT_CHUNKS

_CACHE = {}


def _build(reps=1):
    import contextlib

    import concourse.bacc as bacc
    import concourse.mybir as mybir
    import concourse.tile as tile

    f32 = mybir.dt.float32
    bf16 = mybir.dt.bfloat16
    AOP = mybir.AluOpType
    Square = mybir.ActivationFunctionType.Square

    nc = bacc.Bacc(None, target_bir_lowering=False)

    xt = nc.declare_dram_parameter("xt", [NT, 128, NCHUNK * TILE], bf16,
                                   isOutput=False)
    w1d = nc.declare_dram_parameter("w1d", [128, NCHUNK, O1], bf16,
                                    isOutput=False)
    w2d = nc.declare_dram_parameter("w2d", [128, O2], bf16, isOutput=False)
    w3d = nc.declare_dram_parameter("w3d", [128, O3], bf16, isOutput=False)
    b1d = nc.declare_dram_parameter("b1d", [128, 1], f32, isOutput=False)
    b2d = nc.declare_dram_parameter("b2d", [128, 1], f32, isOutput=False)
    b3d = nc.declare_dram_parameter("b3d", [128, 1], f32, isOutput=False)
    outt = nc.declare_dram_parameter("outt", [O3, BC], f32, isOutput=True)

    ND, NA = N_DVE_CHUNKS, N_ACT_CHUNKS

    with tile.TileContext(nc) as tc:
        with (
            tc.tile_pool(name="const", bufs=1) as cpool,
            tc.tile_pool(name="xp", bufs=XBUFS) as xpool,
            tc.tile_pool(name="sqp", bufs=4) as sqpool,
            tc.tile_pool(name="qap", bufs=4) as qapool,
            tc.tile_pool(name="sb", bufs=2) as spool,
            tc.tile_pool(name="psA", bufs=2, space="PSUM") as pA,
            tc.tile_pool(name="psB", bufs=2, space="PSUM") as pB,
            tc.tile_pool(name="psS", bufs=1, space="PSUM") as pS,
        ):
            # ------------- constants (host-precomputed weights) -------------
            lhsT1 = cpool.tile([128, NCHUNK, O1], bf16)
            nc.gpsimd.dma_start(out=lhsT1, in_=w1d[:])
            lhsT2 = cpool.tile([128, O2], bf16)
            nc.gpsimd.dma_start(out=lhsT2, in_=w2d[:])
            lhsT3 = cpool.tile([128, O3], bf16)
            nc.gpsimd.dma_start(out=lhsT3, in_=w3d[:])
            bias1 = cpool.tile([128, 1], f32)
            nc.gpsimd.dma_start(out=bias1, in_=b1d[:])
            bias2 = cpool.tile([128, 1], f32)
            nc.gpsimd.dma_start(out=bias2, in_=b2d[:])
            bias3 = cpool.tile([128, 1], f32)
            nc.gpsimd.dma_start(out=bias3, in_=b3d[:])

            onesP1 = cpool.tile([128, O1], bf16)
            nc.vector.memset(onesP1, S1_EFF)
            onesN1 = cpool.tile([128, O1], bf16)
            nc.vector.memset(onesN1, -S1_EFF)
            onesB2 = cpool.tile([128, O2], bf16)
            nc.vector.memset(onesB2, -S2)
            onesB3 = cpool.tile([128, O3], bf16)
            nc.vector.memset(onesB3, -S3)
            biasq = cpool.tile([128, 1], f32)
            nc.vector.memset(biasq, -0.5)

            # ---------------- main loop (3-stage software pipeline) ---------
            state = {}   # per-pair tiles carried to the tail stage
            qstate = {}  # per-quad psum tiles carried to the quad stage

            def head(pr):
                """DMA + s1-matmuls + sq/q elementwise for pair pr."""
                t0, t1 = 2 * pr, 2 * pr + 1
                xs = []
                for tt in (t0, t1):
                    x_t = xpool.tile([128, NCHUNK * TILE], bf16, tag="x",
                                     name=f"x_{tt}")
                    nc.sync.dma_start(out=x_t, in_=xt[tt])
                    xs.append(x_t)
                psA1 = pA.tile([128, TILE], f32, tag="A1", name=f"psA1_{pr}")
                psB1 = pB.tile([128, TILE], f32, tag="B1", name=f"psB1_{pr}")
                # interleaved s1 matmuls: the two tiles alternate column
                # groups every instruction -> concurrent in the PE array
                for c in range(NCHUNK):
                    for k in range(2):
                        nc.tensor.matmul(
                            psA1[O1 * k : O1 * (k + 1), :],
                            lhsT1[:, c, :],
                            xs[k][:, c * TILE : (c + 1) * TILE],
                            start=(c == 0),
                            stop=(c == NCHUNK - 1),
                            tile_position=(0, O1 * k),
                        )
                sqs, qas = [], []
                for k, tt in enumerate((t0, t1)):
                    sq = sqpool.tile([128, ND * TILE], bf16, tag="sq",
                                     name=f"sq_{tt}")
                    nc.vector.tensor_mul(
                        sq, xs[k][:, 0 : ND * TILE], xs[k][:, 0 : ND * TILE]
                    )
                    sqs.append(sq)
                    qa = qapool.tile([128, NA * TILE], bf16, tag="qa",
                                     name=f"qa_{tt}")
                    nc.scalar.activation(
                        qa, xs[k][:, ND * TILE :], Square, bias=biasq,
                        scale=1.0,
                    )
                    qas.append(qa)
                state[pr] = (xs, sqs, qas, psA1, psB1)

            def tailB(pr):
                """C-term matmuls + layer-1 combine + layer-2 MMs, pair pr."""
                t1 = 2 * pr + 1
                qj = t1 % 4
                xs, sqs, qas, psA1, psB1 = state.pop(pr)
                # C-term streams, interleaved across the two tiles:
                #   +s * x (chunks 0..2), -s * x^2, -s * (x-0.5)^2
                nmm = ND * 2 + NA
                i = 0
                for src_list, ones in (
                    (xs, onesP1),
                    (sqs, onesN1),
                    (qas, onesN1),
                ):
                    nch = ND if src_list is not qas else NA
                    for c in range(nch):
                        for k in range(2):
                            nc.tensor.matmul(
                                psB1[O1 * k : O1 * (k + 1), :],
                                ones,
                                src_list[k][:, c * TILE : (c + 1) * TILE],
                                start=(i == 0),
                                stop=(i == nmm - 1),
                                tile_position=(0, O1 * k),
                            )
                        i += 1
                # layer-1 combine for the pair -> p2 [128, 512]
                t2p = spool.tile([128, TILE], f32, tag="t2", name=f"t2_{pr}")
                nc.scalar.activation(
                    t2p, psA1, Square, bias=bias1, scale=2.0 / L1
                )
                p2p = spool.tile([128, TILE], bf16, tag="p2", name=f"p2_{pr}")
                nc.vector.scalar_tensor_tensor(
                    p2p, t2p, K1, psB1, AOP.add, AOP.add
                )
                # layer 2 for both tiles of the pair
                q2p = spool.tile([128, TILE], bf16, tag="q2", name=f"q2_{pr}")
                nc.vector.scalar_tensor_tensor(
                    q2p, p2p, 1.0, p2p, AOP.subtract, AOP.mult
                )
                if qj == 1:
                    psA2 = pA.tile([128, TILE], f32, tag="A2", bufs=1,
                                   name=f"psA2_{pr}")
                    psB2 = pB.tile([128, TILE], f32, tag="B2", bufs=1,
                                   name=f"psB2_{pr}")
                    qstate[pr // 2] = (psA2, psB2)
                else:
                    psA2, psB2 = qstate[pr // 2]
                for half, tq in ((0, qj - 1), (1, qj)):
                    hs = slice(O1 * half, O1 * (half + 1))
                    tp = (O1 * half, O2 * tq)
                    nc.tensor.matmul(
                        psA2[O2 * tq : O2 * (tq + 1), :],
                        lhsT2[hs, :],
                        p2p[hs, :],
                        tile_position=tp,
                    )
                    nc.tensor.matmul(
                        psB2[O2 * tq : O2 * (tq + 1), :],
                        onesB2[hs, :],
                        q2p[hs, :],
                        tile_position=tp,
                    )

            def quad_pre(Q):
                """Layer-2 combine part 1 (ACT) for quad Q."""
                psA2, psB2 = qstate[Q]
                tq2 = spool.tile([128, TILE], f32, tag="tq2", name=f"tq2_{Q}")
                nc.scalar.activation(
                    tq2, psA2, Square, bias=bias2, scale=2.0 / L2
                )
                qstate[Q] = (psA2, psB2, tq2)

            def quad_mid(Q):
                """Layer-2 combine part 2 + layer 3 + output for quad Q."""
                _, psB2, tq2 = qstate.pop(Q)
                p3q = spool.tile([128, TILE], bf16, tag="p3", name=f"p3_{Q}")
                nc.vector.scalar_tensor_tensor(
                    p3q, tq2, 0.0, psB2, AOP.add, AOP.add
                )
                q3q = spool.tile([128, TILE], bf16, tag="q3", name=f"q3_{Q}")
                nc.vector.scalar_tensor_tensor(
                    q3q, p3q, 1.0, p3q, AOP.subtract, AOP.mult
                )
                psA3 = pS.tile([128, TILE], f32, tag="A3", name=f"psA3_{Q}")
                psB3 = pS.tile([128, TILE], f32, tag="B3", name=f"psB3_{Q}")
                for j in range(4):
                    js = slice(32 * j, 32 * j + L3)
                    tp = (32 * j, 32 * j)
                    nc.tensor.matmul(
                        psA3[32 * j : 32 * j + O3, :],
                        lhsT3[js, :],
                        p3q[js, :],
                        tile_position=tp,
                    )
                    nc.tensor.matmul(
                        psB3[32 * j : 32 * j + O3, :],
                        onesB3[js, :],
                        q3q[js, :],
                        tile_position=tp,
                    )
                t3q = spool.tile([128, TILE], f32, tag="t3", name=f"t3_{Q}")
                nc.scalar.activation(
                    t3q, psA3, Square, bias=bias3, scale=2.0 / L3
                )
                outq = spool.tile([128, TILE], f32, tag="outq",
                                  name=f"outq_{Q}")
                nc.vector.scalar_tensor_tensor(
                    outq, t3q, 0.0, psB3, AOP.add, AOP.add
                )
                for j in range(4):
                    tt = 4 * Q + j
                    nc.gpsimd.dma_start(
                        out=outt[:, tt * TILE : (tt + 1) * TILE],
                        in_=outq[32 * j : 32 * j + O3, :],
                    )

            loop_cm = (
                tc.For_i(0, reps, 1) if reps > 1 else contextlib.nullcontext()
            )
            with loop_cm:
                for it in range(NPAIR + 2):
                    Q = (it - 3) // 2
                    do_quad = it >= 3 and (it - 3) % 2 == 0 and Q < NPAIR // 2
                    if do_quad:
                        quad_pre(Q)
                    if it < NPAIR:
                        head(it)
                    if do_quad:
                        quad_mid(Q)
                    if 1 <= it <= NPAIR:
                        tailB(it - 1)

    nc.compile()
    return nc


def _get_nc(reps=1):
    key = ("nc", reps)
    if key not in _CACHE:
        _CACHE[key] = _build(reps)
    return _CACHE[key]


def _make_in_maps(x, w1, w2, w3):
    x = np.asarray(x, dtype=np.float32)
    w1 = np.asarray(w1, dtype=np.float32)
    w2 = np.asarray(w2, dtype=np.float32)
    w3 = np.asarray(w3, dtype=np.float32)

    # binarized weights and their column sums (all tiny -> host)
    wb1 = np.where(w1 >= 0, 1.0, -1.0).astype(np.float32)   # [64, 768]
    wb2 = np.where(w2 >= 0, 1.0, -1.0).astype(np.float32)   # [32, 64]
    wb3 = np.where(w3 >= 0, 1.0, -1.0).astype(np.float32)   # [4, 32]

    # lhsT1[p, c, o] = wb1[o, c*128+p]
    w1d = np.ascontiguousarray(
        wb1.T.reshape(NCHUNK, 128, O1).transpose(1, 0, 2)
    ).astype(BF16)
    # lhsT2: [128, 32], rows 0..63 = wb2.T, rows 64..127 = copy
    w2d = np.ascontiguousarray(np.tile(wb2.T, (2, 1))).astype(BF16)
    # lhsT3: [128, 4], wb3.T replicated 4x
    w3d = np.ascontiguousarray(np.tile(wb3.T, (4, 1))).astype(BF16)

    c1 = wb1.sum(axis=1)   # [64]
    c2 = wb2.sum(axis=1)   # [32]
    c3 = wb3.sum(axis=1)   # [4]
    b1d = np.ascontiguousarray(
        np.tile(-c1 / L1, 2).reshape(128, 1)
    ).astype(np.float32)
    b2d = np.ascontiguousarray(
        np.tile(-c2 / L2, 4).reshape(128, 1)
    ).astype(np.float32)
    b3 = np.zeros((4, 32), np.float32)
    b3[:, :O3] = -c3 / L3
    b3d = np.ascontiguousarray(b3.reshape(128, 1))

    xs = x.reshape(NCORES, NT, TILE, NCHUNK, 128)
    # [core][tile, partition(f%128), chunk(f//128)*TILE + batch-in-tile], bf16
    xtiled = np.ascontiguousarray(
        xs.transpose(0, 1, 4, 3, 2).astype(BF16)
    ).reshape(NCORES, NT, 128, NCHUNK * TILE)
    return [
        {
            "xt": xtiled[i],
            "w1d": w1d,
            "w2d": w2d,
            "w3d": w3d,
            "b1d": b1d,
            "b2d": b2d,
            "b3d": b3d,
        }
        for i in range(NCORES)
    ]


def kernel(x, w1, w2, w3):
    from concourse.bass_utils import run_bass_kernel_spmd

    nc = _get_nc()
    in_maps = _make_in_maps(x, w1, w2, w3)
    res = run_bass_kernel_spmd(nc, in_maps, core_ids=list(range(NCORES)))
    return np.concatenate(
        [res.results[i]["outt"].T for i in range(NCORES)], axis=0
    ).astype(np.float32)


def bench(x, w1, w2, w3, iters=20, reps=1, cores=NCORES):
    """Time device execution with a persistent jit and device-resident
    inputs (excludes host<->device transfer and compile).  Returns
    (output, per_call_seconds_list).  NOTE: per-call wall time under axon
    is dominated by a fixed ~80ms relay dispatch latency; use the NTFF
    profile (run_bass_kernel_spmd(trace=True)) for true HW exec time."""
    import time

    import jax
    from jax.sharding import Mesh, NamedSharding, PartitionSpec
    from jax.experimental.shard_map import shard_map

    import concourse.mybir as mybir
    from concourse import bass2jax
    from concourse.bass2jax import _bass_exec_p, install_neuronx_cc_hook

    nc = _get_nc(reps)
    install_neuronx_cc_hook()
    in_maps = _make_in_maps(x, w1, w2, w3)

    partition_name = (
        nc.partition_id_tensor.name if nc.partition_id_tensor else None
    )
    in_names, out_names, out_avals, zero_outs = [], [], [], []
    for alloc in nc.m.functions[0].allocations:
        if not isinstance(alloc, mybir.MemoryLocationSet):
            continue
        name = alloc.memorylocations[0].name
        if alloc.kind == "ExternalInput":
            if name != partition_name:
                in_names.append(name)
        elif alloc.kind == "ExternalOutput":
            out_names.append(name)
            shape = tuple(alloc.tensor_shape)
            dtype = mybir.dt.np(alloc.dtype)
            out_avals.append(jax.core.ShapedArray(shape, dtype))
            zero_outs.append(np.zeros(shape, dtype))
    n_params = len(in_names)
    in_names = in_names + out_names
    if partition_name is not None:
        in_names = in_names + [partition_name]

    def _body(*args):
        operands = list(args)
        if partition_name is not None:
            operands.append(bass2jax.partition_id_tensor())
        outs = _bass_exec_p.bind(
            *operands,
            out_avals=tuple(out_avals),
            in_names=tuple(in_names),
            out_names=tuple(out_names),
            lowering_input_output_aliases=(),
            sim_require_finite=True,
            sim_require_nnan=True,
            nc=nc,
        )
        return tuple(outs)

    devices = jax.devices()[:cores]
    mesh = Mesh(np.asarray(devices), ("core",))
    in_specs = (PartitionSpec("core"),) * (n_params + len(out_names))
    out_specs = (PartitionSpec("core"),) * len(out_names)
    fn = jax.jit(
        shard_map(_body, mesh=mesh, in_specs=in_specs, out_specs=out_specs,
                  check_rep=False),
        keep_unused=True,
    )
    sh = NamedSharding(mesh, PartitionSpec("core"))
    dev_in = [
        jax.device_put(
            np.concatenate([in_maps[c][nm] for c in range(cores)], axis=0), sh
        )
        for nm in in_names[:n_params]
    ]
    dev_zero = [
        jax.device_put(
            np.zeros((cores * z.shape[0], *z.shape[1:]), z.dtype), sh
        )
        for z in zero_outs
    ]
    out = fn(*dev_in, *dev_zero)
    jax.block_until_ready(out)
    times = []
    for _ in range(iters):
        t0 = time.perf_counter()
        out = fn(*dev_in, *dev_zero)
        jax.block_until_ready(out)
        times.append(time.perf_counter() - t0)
    out_np = np.asarray(out[0]).reshape(cores, *out_avals[0].shape)
    result = np.concatenate([out_np[c].T for c in range(cores)], axis=0)
    return result.astype(np.float32), times
